# revision 8
# baseline (speedup 1.0000x reference)
"""BirthDeathAttention kernel v2 for 8 Trainium2 NeuronCores.

Math note: persistence_bias ([1,H,1,1]) and importance*0.1 ([B,1,N,1]) are
constant along the softmax key axis, so they cancel inside the softmax.
Plain MHA + output projection remains.

Sharding: core = (batch b, head-group g), 4 heads/core; host sums the 4
partial output projections per batch and adds b_proj.

v2 design (trace-driven, from the 259us baseline):
  The kernel is SCALAR-BOUND: softmax exp is 16.8M elements/core and only
  ScalarE has an exp path (1 elem/lane/cycle @1.2GHz, ~475ns/inst
  overhead) -> ~170us for the 128 [128,1024] ACTIVATEs.  Everything else
  hides under a gapless exp stream:
  - a single 128-slot software pipeline over units t=(block, key-tile):
    S-pair(t+2) -> exp(t) -> one filler item -> U-pair(t-8)
  - S-pairs depend only on the q/k chains, so scalar never waits on
    V/U/normalize/projection work; U lags 8 units (epool holds the gap)
    which also absorbs the normalize DMA round-trip at block boundaries
  - PSUM: psS 2x[128,1024] (4 banks) + psU 3x[65,512] (3) + one shared
    1-bank work pool for qkv-projection chains and output-projection
    accumulators (they are time-disjoint) = 8 banks exactly
  - input DMAs: 7 big descriptors on 3 queues; first exp at ~7us
  - denominators via the ones-column of v (row 64 of U accumulators);
    reciprocal_approx_fast directly on the PSUM sum rows; partition
    broadcast via DRAM round trip on the gpsimd queue
  - projection for query-block nqb spread through the block after
    (pair1, nqb); the last one runs in the tail
"""

import sys

if "/opt/trn_rl_repo" not in sys.path:
    sys.path.insert(0, "/opt/trn_rl_repo")

import numpy as np
import ml_dtypes

import concourse.bass as bass
import concourse.mybir as mybir
import concourse.tile as tile
from concourse.bass_utils import run_bass_kernel_spmd

DIM = 1024
N = 2048
B = 2
HEADS = 16
HEAD_DIM = 64
SCALE = HEAD_DIM ** -0.5
HPG = 4
GC = HPG * HEAD_DIM  # 256
BF16 = mybir.dt.bfloat16
F32 = mybir.dt.float32

KT = DIM // 128   # 8 contraction tiles over model dim
NB = N // 512     # 4 query blocks
NKT = N // 128    # 16 key tiles
# U-pair lag behind exp, per block (tapered): block b's U(j) runs at slot
# 16b + j + LAG_B[b].  LAG_B[b+1] = LAG_B[b] - 1 makes the U stream exactly
# contiguous across block boundaries.  A large lag on block 0 moves the
# b_chain deadlines late enough that V-projection work fits the per-slot
# spare budget instead of starving the exp stream.
LAG_B = [24 - b for b in range(8)]


def _split_multi_waits(nc, max_waits=1):
    """Walrus build here accepts at most one sync-wait per instruction;
    hoist extras onto single-wait NOPs (engine-order equivalent)."""
    uid = [0]
    for f in nc.m.functions:
        for bb in f.blocks:
            insts = bb.instructions
            new = []
            changed = False
            for inst in insts:
                si = inst.sync_info
                if si is not None and len(si.on_wait) > max_waits:
                    waits = list(si.on_wait)
                    for w in waits[:-max_waits]:
                        nop = mybir.InstNoOp(
                            name=f"I-splitw-{uid[0]}", ins=[], outs=[])
                        uid[0] += 1
                        nop.engine = inst.engine
                        nop.sync_info = mybir.SyncInfo(
                            on_wait=[w], on_update=[])
                        new.append(nop)
                    si.on_wait = waits[-max_waits:]
                    inst.sync_info = si
                    changed = True
                new.append(inst)
            if changed:
                bb.instructions = new


def build_core_kernel() -> bass.Bass:
    nc = bass.Bass()
    xT = nc.declare_dram_parameter("xT", [DIM, N], BF16, isOutput=False)
    wqk = nc.declare_dram_parameter("wqk", [DIM, 2 * GC], BF16, isOutput=False)
    wv = nc.declare_dram_parameter("wv", [DIM, GC], BF16, isOutput=False)
    wp = nc.declare_dram_parameter("wp", [GC, DIM], BF16, isOutput=False)
    out = nc.declare_dram_parameter("out", [N, DIM], BF16, isOutput=True)

    xT_r = xT.rearrange("(kt p) n -> p kt n", p=128)
    wqk_r = wqk.rearrange("(kt p) c -> p kt c", p=128)
    wv_r = wv.rearrange("(kt p) c -> p kt c", p=128)
    wp_r = wp.rearrange("(pair p) c -> p pair c", p=128)

    with tile.TileContext(nc) as tc:
        from contextlib import ExitStack

        with ExitStack() as ctx:
            consts = ctx.enter_context(tc.tile_pool(name="consts", bufs=1))
            sbuf = ctx.enter_context(tc.tile_pool(name="sbuf", bufs=1))

            xT_sb = sbuf.tile([128, KT, N], BF16, tag="xT")
            wqk_sb = consts.tile([128, KT, 2 * GC], BF16, tag="wqk")
            wv_sb = consts.tile([128, KT, GC], BF16, tag="wv")
            wp_sb = consts.tile([128, 2, DIM], BF16, tag="wp")
            qk_sb = sbuf.tile([128, 4, N], BF16, tag="qk")
            # v with a ones column per head (stride 65): attention@v then
            # also emits the softmax denominator as row 64
            v_sb = sbuf.tile([128, NKT, HPG * 65], BF16, tag="v")
            o_sb = sbuf.tile([128, 2, N], BF16, tag="o")

            nc.vector.memset(v_sb[:], 1.0)

            # input DMAs: critical-path first.  Only the first-wave set
            # (k01/q01 weight slices + xT query-block 0) transfers
            # immediately; the remaining xT blocks are gated behind tiny
            # marker copies that depend on the first-wave regions, so the
            # critical 1.5MB isn't bandwidth-shared with the other 4.5MB.
            nc.sync.dma_start(
                out=wqk_sb[:, :, 256:384], in_=wqk_r[:, :, 256:384])  # k01
            nc.sync.dma_start(
                out=wqk_sb[:, :, 0:128], in_=wqk_r[:, :, 0:128])      # q01
            nc.gpsimd.dma_start(
                out=xT_sb[:, :, 0:512], in_=xT_r[:, :, 0:512])
            # gates: a 2-column marker copy into each later region, reading
            # the previous region -> each big DMA (write-after-write on its
            # marker columns) starts only once its predecessor has landed,
            # keeping HBM bandwidth on the critical transfer
            nc.gpsimd.tensor_copy(
                xT_sb[:, 0, 512:514], xT_sb[:, 0, 0:2])
            nc.scalar.dma_start(
                out=xT_sb[:, :, 512:1024], in_=xT_r[:, :, 512:1024])
            nc.gpsimd.tensor_copy(
                xT_sb[:, 0, 1024:1026], xT_sb[:, 0, 512:514])
            nc.gpsimd.dma_start(
                out=xT_sb[:, :, 1024:1536], in_=xT_r[:, :, 1024:1536])
            nc.gpsimd.tensor_copy(
                xT_sb[:, 0, 1536:1538], xT_sb[:, 0, 1024:1026])
            nc.scalar.dma_start(
                out=xT_sb[:, :, 1536:2048], in_=xT_r[:, :, 1536:2048])
            nc.gpsimd.tensor_copy(
                wv_sb[:, 0, 0:2], wqk_sb[:, 0, 256:258])
            nc.sync.dma_start(out=wv_sb[:], in_=wv_r[:])
            nc.sync.dma_start(
                out=wqk_sb[:, :, 384:512], in_=wqk_r[:, :, 384:512])  # k23
            nc.sync.dma_start(
                out=wqk_sb[:, :, 128:256], in_=wqk_r[:, :, 128:256])  # q23
            nc.gpsimd.tensor_copy(
                wp_sb[:, 0, 0:2], wv_sb[:, 0, 0:2])
            nc.sync.dma_start(out=wp_sb[:], in_=wp_r[:])

            with (
                tc.tile_pool(name="psS", bufs=2, space="PSUM") as psS,
                tc.tile_pool(name="psU", bufs=3, space="PSUM") as psU,
                tc.tile_pool(name="psW", bufs=1, space="PSUM") as psW,
                tc.tile_pool(name="epool", bufs=LAG_B[0] + 3) as epool,
                tc.tile_pool(name="rpool", bufs=2) as rpool,
                tc.tile_pool(name="rdram", bufs=2, space="DRAM") as rdram,
                tc.tile_pool(name="opool", bufs=3) as opool,
            ):
                # ---- emitters -----------------------------------------
                def a_chain(ct, nb, lo=0, hi=512):
                    acc = psW.tile([128, 512], F32, tag="w")
                    for kt in range(KT):
                        nc.tensor.matmul(
                            acc[:, 0:hi - lo],
                            lhsT=wqk_sb[:, kt, ct * 128:(ct + 1) * 128],
                            rhs=xT_sb[:, kt,
                                      nb * 512 + lo:nb * 512 + hi],
                            start=(kt == 0),
                            stop=(kt == KT - 1),
                        )
                    nc.vector.tensor_copy(
                        qk_sb[:, ct, nb * 512 + lo:nb * 512 + hi],
                        acc[:, 0:hi - lo])

                def b_chain(nt):
                    acc = psW.tile([128, 512], F32, tag="w")
                    for kt in range(KT):
                        nc.tensor.matmul(
                            acc[:, 0:GC],
                            lhsT=xT_sb[:, kt, nt * 128:(nt + 1) * 128],
                            rhs=wv_sb[:, kt, :],
                            start=(kt == 0),
                            stop=(kt == KT - 1),
                        )
                    # one strided copy into the 65-stride v layout (the
                    # ones columns were preset by the memset)
                    nc.vector.tensor_copy(
                        v_sb[:, nt, :].rearrange(
                            "p (h c) -> p h c", h=HPG)[:, :, 0:64],
                        acc[:, 0:GC].rearrange("p (h c) -> p h c", h=HPG),
                    )

                def proj_acc(mt, nh, acc=None, evict_scalar=False):
                    # acc override: the tail passes half-tiles of the
                    # retired psS pool so four accumulators ping-pong
                    # instead of serializing on the single psW bank
                    if acc is None:
                        acc = psW.tile([128, 512], F32, tag="w")
                    for pair in range(2):
                        nc.tensor.matmul(
                            acc[:],
                            lhsT=o_sb[:, pair, mt * 128:(mt + 1) * 128],
                            rhs=wp_sb[:, pair, nh * 512:(nh + 1) * 512],
                            start=(pair == 0),
                            stop=(pair == 1),
                        )
                    ot = opool.tile([128, 512], BF16, tag="ot")
                    if evict_scalar:
                        # tail only: scalar queue is idle once exps end
                        nc.scalar.copy(ot[:], acc[:])
                    else:
                        nc.vector.tensor_copy(ot[:], acc[:])
                    nc.sync.dma_start(
                        out=out[mt * 128:(mt + 1) * 128,
                                nh * 512:(nh + 1) * 512],
                        in_=ot[:])

                def s_pair(pair, nqb, j):
                    qt = qk_sb[:, pair, :]
                    kt_sb = qk_sb[:, 2 + pair, :]
                    st = psS.tile([128, 1024], F32, tag="st")
                    for hh in range(2):
                        nc.tensor.matmul(
                            st[:, hh * 512:(hh + 1) * 512],
                            lhsT=kt_sb[
                                hh * 64:(hh + 1) * 64,
                                j * 128:(j + 1) * 128],
                            rhs=qt[
                                hh * 64:(hh + 1) * 64,
                                nqb * 512:(nqb + 1) * 512],
                            start=True,
                            stop=True,
                        )
                    return st

                def exp_unit(st):
                    e_t = epool.tile([128, 1024], BF16, tag="e")
                    nc.scalar.activation(
                        e_t[:], st[:],
                        mybir.ActivationFunctionType.Exp, scale=SCALE)
                    return e_t

                def u_pair(u_ab, pair, j, e_t):
                    for hh in range(2):
                        h = pair * 2 + hh
                        nc.tensor.matmul(
                            u_ab[hh][:],
                            lhsT=v_sb[:, j, h * 65:h * 65 + 65],
                            rhs=e_t[:, hh * 512:(hh + 1) * 512],
                            start=(j == 0),
                            stop=(j == NKT - 1),
                        )

                def u_norm(pair, nqb, u_ab):
                    # Copy U out of PSUM immediately (frees the psU slots
                    # ~1 slot after the last u_pair, well before the next
                    # block's rotation reuses them); the recip/broadcast/
                    # multiply chain then runs from SBUF off the PE path.
                    u_a, u_b = u_ab
                    usb = rpool.tile([128, 512], F32, tag="usb")
                    rs = rpool.tile([33, 512], F32, tag="rs")
                    r_t = rpool.tile([33, 512], F32, tag="r")
                    rr_t = rpool.tile([128, 512], F32, tag="rr")
                    r_dr = rdram.tile([2, 512], F32, tag="rdr")
                    nc.vector.tensor_copy(usb[0:64, :], u_a[0:64, :])
                    nc.vector.tensor_copy(usb[64:128, :], u_b[0:64, :])
                    nc.vector.tensor_copy(rs[0:1, :], u_a[64:65, :])
                    nc.vector.tensor_copy(rs[32:33, :], u_b[64:65, :])
                    # rows 1..31 hold garbage; only rows 0/32 are consumed.
                    # plain reciprocal (~4us) is fine here: the whole chain
                    # sits in block-boundary slack, off the exp/PE path.
                    nc.vector.reciprocal(r_t[0:33, :], rs[0:33, :])
                    for hh in range(2):
                        nc.gpsimd.dma_start(
                            out=r_dr[hh:hh + 1, :],
                            in_=r_t[hh * 32:hh * 32 + 1, :])
                        nc.gpsimd.dma_start(
                            out=rr_t[hh * 64:(hh + 1) * 64, :],
                            in_=r_dr[hh:hh + 1, :].to_broadcast([64, 512]))
                    nc.vector.tensor_mul(
                        o_sb[:, pair, nqb * 512:(nqb + 1) * 512],
                        usb[:, :], rr_t[:, :])

                # ---- unified slot pipeline with budget-paced filler ---
                blocks = [(p, q) for p in range(2) for q in range(NB)]
                NU = len(blocks) * NKT  # 128 units

                def unit(t):
                    return blocks[t // NKT], t % NKT

                # filler items: (kind, a1, a2, deadline_slot, est_cost_us).
                # a-chains gate S waves (hard deadlines); b-chains gate
                # block 0's U pairs at slot nt + LAG_B[0].
                # NOTE deadlines: S(t+2) is emitted BEFORE emit_slot_work(t)
                # each slot, so a chain consumed by the S-pair of slot t
                # must carry deadline <= t-1 (same-slot emission would put
                # it AFTER the consuming S in PE program order -- a
                # read-before-write that Tile sees as no dependency at all:
                # the S matmul silently races the qk copy).
                filler = (
                    [("k", 2, 0, 0, 1.4),  # k01 keys 128:512 remainder
                     ("a", 2, 2, 5, 1.8),
                     ("a", 2, 3, 9, 1.8), ("a", 0, 1, 13, 1.8)]
                    + [("b", nt, 0, nt + LAG_B[0] - 1, 1.0)
                       for nt in range(NKT)]
                    + [("a", 0, 2, 29, 1.8), ("a", 0, 3, 45, 1.8)]
                    + [("a", 3, 0, 58, 1.8), ("a", 3, 1, 62, 1.8),
                       ("a", 3, 2, 66, 1.8), ("a", 3, 3, 70, 1.8)]
                    + [("a", 1, 0, 61, 1.8), ("a", 1, 1, 77, 1.8),
                       ("a", 1, 2, 93, 1.8), ("a", 1, 3, 109, 1.8)]
                )
                filler.sort(key=lambda it: it[3])
                fill_i = [0]
                spent = [0.0]
                proj_q = []
                SPARE_PER_SLOT = 0.38  # us of filler budget per slot

                def emit_one_filler():
                    kind, a1, a2, _, cost = filler[fill_i[0]]
                    fill_i[0] += 1
                    spent[0] += cost
                    if kind == "a":
                        a_chain(a1, a2)
                    elif kind == "k":
                        a_chain(a1, a2, 128, 512)
                    else:
                        b_chain(a1)

                proj_spent = [0.0]
                # keep in-stream projection light: the late a-chains and a
                # heavy proj flow used to collide in slots 90-120 and open
                # multi-us holes at the end of the exp stream; the tail's
                # 4-deep psS ping-pong absorbs deferred projections cheaply
                PROJ_START = 85   # slot where in-stream proj budget opens
                PROJ_RATE = 0.0   # all projection work goes to the tail

                def emit_slot_work(t):
                    # deadline-forced items first, then budget-paced ones;
                    # projection accumulators have their own budget window
                    # (they only become available late in the stream)
                    while (fill_i[0] < len(filler)
                           and filler[fill_i[0]][3] <= t):
                        emit_one_filler()
                    budget = (t + 1) * SPARE_PER_SLOT
                    while (fill_i[0] < len(filler)
                           and spent[0] + filler[fill_i[0]][4] <= budget):
                        emit_one_filler()
                    pbudget = max(0, t - PROJ_START) * PROJ_RATE
                    if proj_q and proj_spent[0] + 0.55 <= pbudget:
                        mt, nh = proj_q.pop(0)
                        proj_spent[0] += 0.55
                        proj_acc(mt, nh)

                # prologue: partial k chain (keys 0:128 only) + q chain is
                # the minimum gating the first S pair; the k remainder is
                # the first (deadline-0) filler item, after which S(1) runs
                a_chain(2, 0, 0, 128)
                a_chain(0, 0)
                # a(2,1) pre-run here: as a deadline-1 filler it stalled
                # the stream's first slots; before S(0) it costs nothing
                a_chain(2, 1)

                st_q = {}
                e_q = {}
                u_tiles = {}
                tail_halves = []

                def tail_acc():
                    # fresh psS tile every second acc; both halves serve as
                    # independent 1-bank accumulators (pool rotation gives
                    # 4 accs in flight across the 2 retired psS slots)
                    if not tail_halves:
                        st = psS.tile([128, 1024], F32, tag="st")
                        tail_halves.append(st[:, 512:1024])
                        tail_halves.append(st[:, 0:512])
                    return tail_halves.pop()

                st_q[0] = s_pair(*unit(0)[0], 0)

                # u_slot[t] = global units whose U-pairs run at slot t.
                # NOTE: with the -1-per-block lag taper, block b's unit 15
                # and block b+1's unit 0 land on the SAME slot -- the map
                # must hold a list (a plain dict assignment would silently
                # drop one unit per block boundary, losing the u_norm).
                u_slot = {}
                for b in range(len(blocks)):
                    for j in range(NKT):
                        t_u = 16 * b + j + LAG_B[b]
                        u_slot.setdefault(t_u, []).append(16 * b + j)

                T_END = max(u_slot) + 1
                for t in range(T_END):
                    if t < NU:
                        if t == 0:
                            # k01 remainder then S(1): deadline-0 filler
                            emit_one_filler()
                            (p1_, q1_), j1_ = unit(1)
                            st_q[1] = s_pair(p1_, q1_, j1_)
                        e_q[t] = exp_unit(st_q.pop(t))
                        if t + 2 < NU:
                            (p2, q2), j2 = unit(t + 2)
                            st_q[t + 2] = s_pair(p2, q2, j2)
                        emit_slot_work(t)
                    else:
                        # drain region: interleave leftovers with tail U,
                        # projection accumulators ping-pong over retired
                        # psS banks with alternating evict engines
                        for _ in range(3):
                            if fill_i[0] < len(filler):
                                emit_one_filler()
                            elif proj_q:
                                mt, nh = proj_q.pop(0)
                                proj_acc(mt, nh, acc=tail_acc(),
                                         evict_scalar=bool(len(proj_q) % 2))
                    for tu in u_slot.get(t, ()):
                        (pu, qu), ju = unit(tu)
                        bi = tu // NKT
                        if ju == 0:
                            u_a = psU.tile([65, 512], F32, tag="u")
                            u_b = psU.tile([65, 512], F32, tag="u")
                            u_tiles[bi] = (u_a, u_b)
                        u_pair(u_tiles[bi], pu, ju, e_q.pop(tu))
                        if ju == NKT - 1:
                            u_norm(pu, qu, u_tiles.pop(bi))
                            if pu == 1:
                                for mt in range(qu * 4, qu * 4 + 4):
                                    for nh in range(2):
                                        proj_q.append((mt, nh))

                # drain remaining projections
                while proj_q:
                    mt, nh = proj_q.pop(0)
                    proj_acc(mt, nh, acc=tail_acc(),
                             evict_scalar=bool(len(proj_q) % 2))

                assert not u_tiles and not e_q and not st_q, (
                    "pipeline drain incomplete",
                    len(u_tiles), len(e_q), len(st_q))
                assert fill_i[0] == len(filler)

    _split_multi_waits(nc)
    return nc


_NC_CACHE = None


def _get_nc():
    global _NC_CACHE
    if _NC_CACHE is None:
        _NC_CACHE = build_core_kernel()
    return _NC_CACHE


def kernel(x, importance_weights, W_qkv, W_proj, b_proj, persistence_bias,
           _results_hook=None):
    x = np.asarray(x)
    W_qkv = np.asarray(W_qkv, dtype=np.float32)
    W_proj = np.asarray(W_proj, dtype=np.float32)
    b_proj = np.asarray(b_proj, dtype=np.float32)

    bf = ml_dtypes.bfloat16
    Q = W_qkv[:, 0:DIM]
    K = W_qkv[:, DIM:2 * DIM]
    V = W_qkv[:, 2 * DIM:3 * DIM]

    in_maps = []
    for core in range(8):
        b, g = divmod(core, 4)
        sl = slice(g * GC, (g + 1) * GC)
        in_maps.append({
            "xT": np.ascontiguousarray(x[b].T).astype(bf),
            "wqk": np.ascontiguousarray(
                np.concatenate([Q[:, sl], K[:, sl]], axis=1)).astype(bf),
            "wv": np.ascontiguousarray(V[:, sl]).astype(bf),
            "wp": np.ascontiguousarray(W_proj[sl, :]).astype(bf),
        })

    nc = _get_nc()
    res = run_bass_kernel_spmd(nc, in_maps, list(range(8)))
    if _results_hook is not None:
        _results_hook(res)

    out = np.zeros((B, N, DIM), dtype=np.float32)
    for core in range(8):
        b = core // 4
        out[b] += res.results[core]["out"].astype(np.float32)
    out += b_proj[None, None, :]
    return out
